# revision 1
# baseline (speedup 1.0000x reference)
"""Trainium2 Bass kernel for CAM-style channel attention module.

Reference computation (per batch b):
    Q  = W @ X + bias          # 1x1 conv: [256,512]@[512,4096] -> [256,4096]
    E  = Q @ X^T / sqrt(4096)  # [256,512] channel-attention energy
    A  = softmax(E, axis=-1)
    out = gamma * (A @ X) + Q  # residual

Key algebraic trick: the residual never needs Q materialized ——
    gamma*(A@X) + (W@X + b) = (W + gamma*A) @ X + b
so the final stage is a single fused matmul with combined weights.

Device strategy (8 NeuronCores, data-parallel over batch, 2 batches/core).
All matmuls bf16 with fp32 PSUM accumulation. Per batch:
  1. DMA-cast X fp32->bf16 (SWDGE inline cast) in progressive column chunks
     (4x256 then 6x512) so PE can start after the first chunk lands.
  2. Fused main loop over 32 n-tiles: the SAME stationary X-tile feeds
     (a) a transpose as a REGULAR matmul against identity (keeps the PE HAM
         clock-gate warm, unlike transpose-mode) building XT
     (b) the QT matmul (rhs=W^T) building QT = X^T W^T (+b on evacuation)
     and the energy matmuls E[q,:] += QT[n-tile,q]^T XT[n-tile,:] are
     interleaved with a 1-iteration lag so PE never stalls on evacuations.
  3. softmax on ScalarE (exp with fused accum row-sums) / VectorE;
     gamma and 1/rowsum fold into one per-row scale of A.
  4. A^T via regular matmul vs identity; lhsT_final = W^T + scaled-A^T.
  5. final = lhsT_final^T @ X (+b via bias-add on PSUM evacuation,
     alternating ScalarE/VectorE).
"""

import numpy as np
import ml_dtypes

import concourse.bass as bass
import concourse.tile as tile
from concourse import bacc, mybir
from concourse.bass_utils import run_bass_kernel_spmd

P = 128
NB = 2        # batches per core (B=16 over 8 cores)
C = 512       # input channels
C1 = 256      # conv output channels
HW = 4096     # H*W
CT = C // P   # 4 c-tiles
NT = HW // P  # 32 n-tiles
QT = C1 // P  # 2 q-tiles
NCHUNK = 512  # matmul free-dim chunk (one PSUM bank)
# x DMA column chunk widths: small leading chunks so PE starts early
XCHUNKS = [256] * 4 + [512] * 6
XBOUND = [0]
for _w in XCHUNKS:
    XBOUND.append(XBOUND[-1] + _w)
assert XBOUND[-1] == HW
F32 = mybir.dt.float32
BF16 = mybir.dt.bfloat16
SCALE = 1.0 / 64.0  # 1/sqrt(HW)

N_CORES = 8


def build_nc():
    nc = bacc.Bacc("TRN2", target_bir_lowering=False, debug=False,
                   num_devices=N_CORES)

    xs = nc.dram_tensor("xs", [NB, C, HW], F32, kind="ExternalInput").ap()
    wt_f = nc.dram_tensor("wt_f", [P, CT, C1], F32, kind="ExternalInput").ap()
    wt_b = nc.dram_tensor("wt_b", [P, CT, C1], BF16, kind="ExternalInput").ap()
    bbc = nc.dram_tensor("bbc", [P, C1], F32, kind="ExternalInput").ap()
    bq = nc.dram_tensor("bq", [P, QT], F32, kind="ExternalInput").ap()
    gam = nc.dram_tensor("gam", [P, 1], F32, kind="ExternalInput").ap()
    out = nc.dram_tensor("out", [NB, C1, HW], F32, kind="ExternalOutput").ap()

    ident_dram = nc.inline_tensor(np.eye(P, dtype=ml_dtypes.bfloat16),
                                  name="ident")

    with tile.TileContext(nc) as tc:
        with (
            tc.tile_pool(name="const", bufs=1) as const,
            tc.tile_pool(name="xb", bufs=2 * len(XCHUNKS)) as xb_pool,
            tc.tile_pool(name="xtq", bufs=8) as xtq_pool,
            tc.tile_pool(name="sm", bufs=2) as sm_pool,
            tc.tile_pool(name="lhsf", bufs=2) as lhsf_pool,
            tc.tile_pool(name="osb", bufs=3) as osb_pool,
            tc.tile_pool(name="psA", bufs=2, space="PSUM") as psA,
            tc.tile_pool(name="psB", bufs=2, space="PSUM") as psB,
            tc.tile_pool(name="psC", bufs=2, space="PSUM") as psC,
        ):
            # ---- constants (all plain HWDGE loads; host pre-broadcast) ----
            ident = const.tile([P, P], BF16)
            nc.sync.dma_start(out=ident, in_=ident_dram.ap())
            wtb_sb = const.tile([P, CT, C1], BF16)
            nc.sync.dma_start(out=wtb_sb, in_=wt_b)
            wtf_sb = const.tile([P, CT, C1], F32)
            nc.sync.dma_start(out=wtf_sb, in_=wt_f)
            bbc_sb = const.tile([P, C1], F32)
            nc.sync.dma_start(out=bbc_sb, in_=bbc)
            bq_sb = const.tile([P, QT], F32)
            nc.sync.dma_start(out=bq_sb, in_=bq)
            gam_sb = const.tile([P, 1], F32)
            nc.sync.dma_start(out=gam_sb, in_=gam)

            x_r = xs.rearrange("b (ct p) n -> b p ct n", p=P)
            out_r = out.rearrange("b (t p) n -> b p t n", p=P)

            # ================= software-pipelined batch schedule ==========
            # PE order: BC(0) | BC(1)[0:16] AT0 F0[0:2] | BC(1)[16:32]
            #           F0[2:4] AT1 F1 — softmax latencies hide under the
            #           other batch's matmul streams; PE never idles.
            st = [dict() for _ in range(NB)]

            def issue_x_dma(bi):
                xb_ch = []
                for j, w in enumerate(XCHUNKS):
                    cht = xb_pool.tile([P, CT, max(XCHUNKS)], BF16, tag="xb",
                                       name=f"xb_{bi}_{j}")
                    nc.gpsimd.dma_start(
                        out=cht[:, :, :w],
                        in_=x_r[bi][:, :, XBOUND[j]:XBOUND[j + 1]])
                    xb_ch.append(cht)
                st[bi]["xb"] = xb_ch

            def xb_slices(bi, ct, lo, width):
                """Slices covering [lo, lo+width) split at chunk bounds.
                Yields (col_offset_in_request, sbuf_slice)."""
                end = lo + width
                out_sl = []
                for j, w in enumerate(XCHUNKS):
                    clo, chi = XBOUND[j], XBOUND[j + 1]
                    if chi <= lo or clo >= end:
                        continue
                    a, b = max(lo, clo), min(end, chi)
                    out_sl.append(
                        (a - lo,
                         st[bi]["xb"][j][:, ct, a - clo:b - clo]))
                return out_sl

            def xb_slice(bi, ct, lo, width):
                sl = xb_slices(bi, ct, lo, width)
                assert len(sl) == 1
                return sl[0][1]

            def emit_B(bi, nt):
                ps_xt = psA.tile([P, C], F32, tag="xt")
                ps_qt = psA.tile([P, C1], F32, tag="qt")
                for ct in range(CT):
                    xtile = xb_slice(bi, ct, nt * P, P)
                    nc.tensor.matmul(ps_xt[:, ct * P:(ct + 1) * P],
                                     xtile, ident, start=True, stop=True)
                    nc.tensor.matmul(ps_qt, xtile, wtb_sb[:, ct, :],
                                     start=(ct == 0), stop=(ct == CT - 1))
                xt_t = xtq_pool.tile([P, C], BF16, tag="xt_sb")
                qt_t = xtq_pool.tile([P, C1], BF16, tag="qt_sb")
                nc.scalar.copy(out=xt_t, in_=ps_xt)
                nc.vector.tensor_add(out=qt_t, in0=ps_qt, in1=bbc_sb)
                return xt_t, qt_t

            def emit_C(bi, nt, xt_t, qt_t):
                for qi in range(QT):
                    nc.tensor.matmul(
                        st[bi]["ps_e"][qi], qt_t[:, qi * P:(qi + 1) * P],
                        xt_t, start=(nt == 0), stop=(nt == NT - 1))

            LAG = 3  # energy matmuls trail the B stage by LAG n-tiles

            def emit_BC_range(bi, lo, hi):
                # fused B + energy with a LAG-iteration lag so the PE never
                # waits on the ScalarE/VectorE PSUM evacuations
                if lo == 0:
                    st[bi]["ps_e"] = [
                        psB.tile([P, C], F32, tag="e", name=f"ps_e{bi}{qi}")
                        for qi in range(QT)]
                    st[bi]["pend"] = []
                for nt in range(lo, hi):
                    cur = emit_B(bi, nt)
                    st[bi]["pend"].append((nt, cur))
                    if len(st[bi]["pend"]) > LAG:
                        pnt, ptiles = st[bi]["pend"].pop(0)
                        emit_C(bi, pnt, *ptiles)
                if hi == NT:
                    for pnt, ptiles in st[bi]["pend"]:
                        emit_C(bi, pnt, *ptiles)
                    st[bi]["pend"] = []

            def emit_softmax(bi):
                a_scaled = sm_pool.tile([P, QT, C], BF16, tag="a",
                                        name=f"a_scaled{bi}")
                for qi in range(QT):
                    ps_e = st[bi]["ps_e"][qi]
                    mx = sm_pool.tile([P, 1], F32, tag="mx")
                    nc.vector.reduce_max(mx, ps_e,
                                         axis=mybir.AxisListType.X,
                                         negate=True)
                    nbias = sm_pool.tile([P, 1], F32, tag="nb")
                    nc.vector.tensor_scalar_mul(nbias, mx, SCALE)
                    a_f = sm_pool.tile([P, C], F32, tag="af")
                    rs = sm_pool.tile([P, 1], F32, tag="rs")
                    nc.scalar.activation(
                        out=a_f, in_=ps_e,
                        func=mybir.ActivationFunctionType.Exp,
                        bias=nbias, scale=SCALE, accum_out=rs)
                    rc = sm_pool.tile([P, 1], F32, tag="rc")
                    nc.vector.reciprocal(rc, rs)
                    sc = sm_pool.tile([P, 1], F32, tag="sc")
                    nc.vector.tensor_mul(sc, rc, gam_sb)
                    nc.vector.tensor_scalar_mul(a_scaled[:, qi, :], a_f, sc)
                st[bi]["a"] = a_scaled

            def emit_ATcombine(bi):
                lhsf = lhsf_pool.tile([P, CT, C1], BF16, name=f"lhsf{bi}")
                a_scaled = st[bi]["a"]
                for ct in range(CT):
                    ps_at = psA.tile([P, C1], F32, tag="qt")
                    for qi in range(QT):
                        nc.tensor.matmul(
                            ps_at[:, qi * P:(qi + 1) * P],
                            a_scaled[:, qi, ct * P:(ct + 1) * P], ident,
                            start=True, stop=True)
                    nc.vector.tensor_add(
                        out=lhsf[:, ct, :], in0=ps_at, in1=wtf_sb[:, ct, :])
                st[bi]["lhsf"] = lhsf

            def emit_F_group(bi, qi, ng, wide_psum=False):
                lhsf = st[bi]["lhsf"]
                o_sb = osb_pool.tile([P, 4 * NCHUNK], F32, tag="o")
                pcount = 0
                for half in range(2):
                    for sub in range(2 * half, 2 * half + 2):
                        nch = ng * 4 + sub
                        pieces = xb_slices(bi, ct=0, lo=nch * NCHUNK,
                                           width=NCHUNK)
                        # one psum tile + evacuation per contiguous piece
                        # (two parallel accumulation groups must not share
                        # a PSUM bank: start=True zeroes the whole bank)
                        for pj, (off, _) in enumerate(pieces):
                            w = (pieces[pj + 1][0] if pj + 1 < len(pieces)
                                 else NCHUNK) - off
                            # after the BC loops retire, their PSUM banks
                            # are free — rotate through 4 banks in the tail
                            if wide_psum and pcount % 2 == 1:
                                ps_o_w = psA.tile([P, C], F32, tag="xt",
                                                  name="ps_o_w")
                                ps_o = ps_o_w[:, :NCHUNK]
                            else:
                                ps_o = psC.tile([P, NCHUNK], F32, tag="po",
                                                name="ps_o")
                            pcount += 1
                            for ct in range(CT):
                                rhs = xb_slices(
                                    bi, ct, nch * NCHUNK + off, w)[0][1]
                                nc.tensor.matmul(
                                    ps_o[:, :w],
                                    lhsf[:, ct, qi * P:(qi + 1) * P],
                                    rhs,
                                    start=(ct == 0), stop=(ct == CT - 1))
                            oslice = o_sb[:, sub * NCHUNK + off:
                                          sub * NCHUNK + off + w]
                            if (sub + pj) % 2 == 0:
                                nc.scalar.add(out=oslice, in_=ps_o[:, :w],
                                              add=bq_sb[:, qi:qi + 1])
                            else:
                                nc.vector.tensor_scalar_add(
                                    oslice, ps_o[:, :w],
                                    bq_sb[:, qi:qi + 1])
                    nc.sync.dma_start(
                        out=out_r[bi, :, qi,
                                  (ng * 4 + 2 * half) * NCHUNK:
                                  (ng * 4 + 2 * half + 2) * NCHUNK],
                        in_=o_sb[:, 2 * half * NCHUNK:
                                 (2 * half + 2) * NCHUNK])

            # ---- HAM warm-up: ~3.5us of dummy matmuls on the identity while
            # the first x chunk is still in flight, so real matmuls start at
            # 2.4 GHz instead of paying the cold-clock ramp.
            # single accumulating tile => no inter-warmup semaphores; PE
            # streams these back-to-back and trips the HAM busy window.
            ps_w = psC.tile([P, NCHUNK], F32, tag="po", name="warm")
            NWARM = 48
            for wj in range(NWARM):
                nc.tensor.matmul(ps_w[:, :P], ident, ident,
                                 start=(wj == 0), stop=(wj == NWARM - 1))

            # ---- the schedule ----
            issue_x_dma(0)
            issue_x_dma(1)
            emit_BC_range(0, 0, NT)
            emit_softmax(0)
            emit_BC_range(1, 0, NT // 2)
            emit_ATcombine(0)
            emit_F_group(0, 0, 0)
            emit_F_group(0, 0, 1)
            emit_BC_range(1, NT // 2, NT)
            emit_softmax(1)
            emit_F_group(0, 1, 0)
            emit_ATcombine(1)
            emit_F_group(0, 1, 1)
            for qi in range(QT):
                for ng in range(2):
                    emit_F_group(1, qi, ng, wide_psum=True)
    nc.compile()
    return nc


_NC_CACHE = None


def _get_nc():
    global _NC_CACHE
    if _NC_CACHE is None:
        _NC_CACHE = build_nc()
    return _NC_CACHE


def make_in_maps(x, conv_w, conv_b, gamma):
    B = x.shape[0]
    xs_full = np.ascontiguousarray(x.reshape(B, C, HW), dtype=np.float32)
    wm = conv_w.reshape(C1, C).astype(np.float32)
    wt = np.ascontiguousarray(wm.T)                    # [C, C1]
    wt_tiled = np.ascontiguousarray(
        wt.reshape(CT, P, C1).transpose(1, 0, 2))      # [P, CT, C1]
    wtb_tiled = wt_tiled.astype(ml_dtypes.bfloat16)
    b_np = conv_b.astype(np.float32)
    bbc = np.ascontiguousarray(np.broadcast_to(b_np[None, :], (P, C1)))
    bq = np.ascontiguousarray(b_np.reshape(QT, P).T)   # [P, QT]
    gam = np.ascontiguousarray(
        np.broadcast_to(gamma.astype(np.float32).reshape(1, 1), (P, 1)))
    in_maps = []
    for ci in range(N_CORES):
        in_maps.append({
            "xs": np.ascontiguousarray(xs_full[NB * ci:NB * (ci + 1)]),
            "wt_f": wt_tiled,
            "wt_b": wtb_tiled,
            "bbc": bbc,
            "bq": bq,
            "gam": gam,
        })
    return in_maps


def kernel(x, conv_w, conv_b, gamma, trace=False):
    """Full inputs in, full output out. Shards batch over 8 NeuronCores."""
    nc = _get_nc()
    in_maps = make_in_maps(x, conv_w, conv_b, gamma)
    res = run_bass_kernel_spmd(nc, in_maps, core_ids=list(range(N_CORES)),
                               trace=trace)
    outs = [r["out"].reshape(NB, C1, 64, 64) for r in res.results]
    full = np.concatenate(outs, axis=0).astype(np.float32)
    if trace:
        kernel.last_results = res
    return full


kernel.last_results = None



# revision 6
# speedup vs baseline: 1.1379x; 1.1379x over previous
"""Trainium2 Bass kernel for CAM-style channel attention module.

Reference computation (per batch b):
    Q  = W @ X + bias          # 1x1 conv: [256,512]@[512,4096] -> [256,4096]
    E  = Q @ X^T / sqrt(4096)  # [256,512] channel-attention energy
    A  = softmax(E, axis=-1)
    out = gamma * (A @ X) + Q  # residual

Algebraic restructure (this version):
  1. E = (W G + b s^T)/64 with G = X X^T (Gram) and s = X @ 1.
     G is symmetric: only the upper-triangular 128-blocks are computed
     (1280 cols/n-tile instead of 2048); the 6 lower blocks are PE
     transposes of the upper ones. W G runs in fp32 (only 8+2 matmuls)
     so the dominant G diagonal (~4096) does not amplify W rounding.
  2. The host pre-transposes x into an n-partitioned bf16 copy, so the
     Gram contraction over n needs NO on-chip transposes at all (the
     old kernel spent 16k PE cycles/batch transposing X).
  3. Residual never materializes Q:  gamma*(A@X) + (W@X + b)
     = (W + gamma*A) @ X + b, a single fused bf16 matmul stage.
  4. softmax without max-subtraction (|E|/64 <= ~25, exp safe in fp32).

Device strategy: 8 NeuronCores, data-parallel over batch, 2 per core.
PE stream: G(b0) | G(b1) with b0's {lower-T, s-row, WG+bias+softmax,
AT} interleaved | F(b0) with b1's mid-stages interleaved | F(b1).
PSUM budget (8 banks): psG 4 + psE 1 + psT 1 + psF 2.
DMA queues: xt on gpsimd, xb on scalar, consts+output on sync.
"""

import numpy as np
import ml_dtypes

import concourse.bass as bass
import concourse.tile as tile
from concourse import bacc, mybir
from concourse.bass_utils import run_bass_kernel_spmd

P = 128
NB = 2         # batches per core (B=16 over 8 cores)
C = 512        # input channels
C1 = 256       # conv output channels
HW = 4096      # H*W
CT = C // P    # 4 c-tiles
NT = HW // P   # 32 n-tiles
QT = C1 // P   # 2 q-tiles
NCH = 8        # x DMA chunks per tensor (xt: 4 n-tiles each; xb: 512 cols)
F32 = mybir.dt.float32
BF16 = mybir.dt.bfloat16
SCALE = 1.0 / 64.0  # 1/sqrt(HW)

N_CORES = 8


def build_nc():
    nc = bacc.Bacc("TRN2", target_bir_lowering=False, debug=False,
                   num_devices=N_CORES)

    # host-prepped inputs
    xt_d = nc.dram_tensor("xt", [NB, P, NT, C], BF16,
                          kind="ExternalInput").ap()   # x^T: [n-part, nt, c]
    xb_d = nc.dram_tensor("xb", [NB, P, CT, HW], BF16,
                          kind="ExternalInput").ap()   # x:   [c-part, ct, n]
    wt_f = nc.dram_tensor("wt_f", [P, CT, C1], F32,
                          kind="ExternalInput").ap()   # W^T tiled, fp32
    b_row = nc.dram_tensor("b_row", [1, C1], BF16,
                           kind="ExternalInput").ap()  # bias as a row
    bq = nc.dram_tensor("bq", [P, QT], F32, kind="ExternalInput").ap()
    gam = nc.dram_tensor("gam", [P, 1], F32, kind="ExternalInput").ap()
    out = nc.dram_tensor("out", [NB, C1, HW], BF16,
                         kind="ExternalOutput").ap()

    ident_dram = nc.inline_tensor(np.eye(P, dtype=ml_dtypes.bfloat16),
                                  name="ident")

    # upper-tri block list (ci < cj) for the 6 transposed lower blocks
    LOWER = [(ci, cj) for ci in range(CT) for cj in range(ci + 1, CT)]

    with tile.TileContext(nc) as tc:
        with (
            tc.tile_pool(name="const", bufs=1) as const,
            tc.tile_pool(name="xt", bufs=2 * NCH) as xt_pool,
            tc.tile_pool(name="xb", bufs=2 * NCH) as xb_pool,
            tc.tile_pool(name="gsb", bufs=2) as gsb_pool,
            tc.tile_pool(name="gtmp", bufs=2) as gtmp_pool,
            tc.tile_pool(name="sm", bufs=2) as sm_pool,
            tc.tile_pool(name="srow", bufs=2) as srow_pool,
            tc.tile_pool(name="lhsf", bufs=2) as lhsf_pool,
            tc.tile_pool(name="osb", bufs=3) as osb_pool,
            tc.tile_pool(name="psG", bufs=4, space="PSUM") as psG,
            tc.tile_pool(name="psE", bufs=1, space="PSUM") as psE,
            tc.tile_pool(name="psT", bufs=1, space="PSUM") as psT,
            tc.tile_pool(name="psF", bufs=2, space="PSUM") as psF,
        ):
            # ---- constants (sync queue) ----
            ident = const.tile([P, P], BF16)
            nc.sync.dma_start(out=ident, in_=ident_dram.ap())
            wtf_sb = const.tile([P, CT, C1], F32)
            nc.sync.dma_start(out=wtf_sb, in_=wt_f)
            brow_sb = const.tile([1, C1], BF16)
            nc.sync.dma_start(out=brow_sb, in_=b_row)
            bq_sb = const.tile([P, QT], F32)
            nc.sync.dma_start(out=bq_sb, in_=bq)
            gam_sb = const.tile([P, 1], F32)
            nc.sync.dma_start(out=gam_sb, in_=gam)

            out_r = out.rearrange("b (t p) n -> b p t n", p=P)

            st = [dict() for _ in range(NB)]

            # ---- x DMAs. Queue = issuing engine: xt on gpsimd, xb on
            # scalar, so the two streams run on parallel DMA queues.
            # Issue order = arrival order per queue.
            for bi in range(NB):
                st[bi]["xt"] = []
                st[bi]["xb"] = []
            for bi in range(NB):
                for j in range(NCH):
                    t = xt_pool.tile([P, 4, C], BF16, tag="xt",
                                     name=f"xt{bi}_{j}")
                    nc.gpsimd.dma_start(
                        out=t, in_=xt_d[bi][:, 4 * j:4 * (j + 1), :])
                    st[bi]["xt"].append(t)
            for bi in range(NB):
                for j in range(NCH):
                    t = xb_pool.tile([P, CT, C], BF16, tag="xb",
                                     name=f"xb{bi}_{j}")
                    nc.scalar.dma_start(
                        out=t, in_=xb_d[bi][:, :, 512 * j:512 * (j + 1)])
                    st[bi]["xb"].append(t)

            # ---- HAM warm-up: dummy matmuls while first x chunk lands.
            ps_w = psF.tile([P, C], F32, tag="po", name="warm")
            NWARM = 48
            for wj in range(NWARM):
                nc.tensor.matmul(ps_w[:, :P], ident, ident,
                                 start=(wj == 0), stop=(wj == NWARM - 1))

            # ---------------- stage emitters ----------------
            def emit_G_nt(bi, nt):
                if nt == 0:
                    st[bi]["psg"] = [
                        psG.tile([P, C], F32, tag="g", name=f"g{bi}_{ci}")
                        for ci in range(CT)]
                ch = st[bi]["xt"][nt // 4]
                for ci in range(CT):
                    nc.tensor.matmul(
                        st[bi]["psg"][ci][:, ci * P:],
                        ch[:, nt % 4, ci * P:(ci + 1) * P],
                        ch[:, nt % 4, ci * P:],
                        start=(nt == 0), stop=(nt == NT - 1))

            def emit_s_reduce(bi):
                # s4[p, ct] = row-sums of x over n (per xb chunk, then
                # collapse chunks); fp32, cast to bf16 for the transpose
                parts = sm_pool.tile([P, CT, NCH], F32, tag="sp",
                                     name=f"sp{bi}")
                for j in range(NCH):
                    nc.vector.reduce_sum(parts[:, :, j:j + 1],
                                         st[bi]["xb"][j],
                                         axis=mybir.AxisListType.X)
                s4 = sm_pool.tile([P, CT], F32, tag="s4", name=f"s4{bi}")
                nc.vector.reduce_sum(s4, parts, axis=mybir.AxisListType.X)
                s4b = sm_pool.tile([P, CT], BF16, tag="s4b", name=f"s4b{bi}")
                nc.vector.tensor_copy(s4b, s4)
                st[bi]["s4b"] = s4b

            def emit_G_evac(bi):
                # alternate scalar/vector so the psG banks free fast
                gsb = gsb_pool.tile([P, CT, C], F32, name=f"gsb{bi}")
                gtmp = gtmp_pool.tile([P, len(LOWER), P], BF16,
                                      name=f"gt{bi}")
                copies = []
                for ci in range(CT):
                    copies.append((gsb[:, ci, ci * P:],
                                   st[bi]["psg"][ci][:, ci * P:]))
                    for cj in range(ci + 1, CT):
                        k = LOWER.index((ci, cj))
                        copies.append((gtmp[:, k, :],
                                       st[bi]["psg"][ci][:, cj * P:(cj + 1) * P]))
                for i, (dst, src) in enumerate(copies):
                    if i % 2 == 0:
                        nc.scalar.copy(out=dst, in_=src)
                    else:
                        nc.vector.tensor_copy(dst, src)
                st[bi]["gsb"] = gsb
                st[bi]["gtmp"] = gtmp

            def emit_G_lower(bi):
                # lower blocks: Gsb[:, cj, ci-block] = upper(ci, cj)^T
                for k, (ci, cj) in enumerate(LOWER):
                    pst = psT.tile([P, C], F32, tag="t", name=f"glt{bi}")
                    nc.tensor.matmul(pst[:, :P], st[bi]["gtmp"][:, k, :],
                                     ident, start=True, stop=True)
                    dst = st[bi]["gsb"][:, cj, ci * P:(ci + 1) * P]
                    if k % 2 == 0:
                        nc.scalar.copy(out=dst, in_=pst[:, :P])
                    else:
                        nc.vector.tensor_copy(dst, pst[:, :P])

            def emit_srow(bi):
                # s4 [P, CT] -> s_row [1, C]: transpose one column at a
                # time (out [1, 128] lands on partition 0 — engines may
                # not read PSUM starting at partition > 0)
                pst = psT.tile([P, C], F32, tag="t", name=f"s4t{bi}")
                for ci in range(CT):
                    nc.tensor.matmul(pst[:1, ci * P:(ci + 1) * P],
                                     st[bi]["s4b"][:, ci:ci + 1], ident,
                                     start=True, stop=True)
                srow = srow_pool.tile([1, C], BF16, name=f"srow{bi}")
                nc.scalar.copy(out=srow, in_=pst[:1, :])
                st[bi]["srow"] = srow

            def emit_WGq(bi, qi):
                # E[qi] = W G + b (x) s  (fp32 + K=1 matmuls), then
                # softmax straight off the PSUM bank (psE bufs=1: the
                # whole per-qi pipeline completes before the next qi).
                if qi == 0:
                    st[bi]["a"] = sm_pool.tile([P, QT, C], BF16, tag="a",
                                               name=f"a{bi}")
                pse = psE.tile([P, C], F32, tag="e", name=f"e{bi}{qi}")
                for ct in range(CT):
                    nc.tensor.matmul(
                        pse, wtf_sb[:, ct, qi * P:(qi + 1) * P],
                        st[bi]["gsb"][:, ct, :],
                        start=(ct == 0), stop=False)
                nc.tensor.matmul(
                    pse, brow_sb[:, qi * P:(qi + 1) * P], st[bi]["srow"],
                    start=False, stop=True)
                a_f = sm_pool.tile([P, C], BF16, tag="af")
                rs = sm_pool.tile([P, 1], F32, tag="rs")
                nc.scalar.activation(
                    out=a_f, in_=pse,
                    func=mybir.ActivationFunctionType.Exp,
                    scale=SCALE, accum_out=rs)
                rc = sm_pool.tile([P, 1], F32, tag="rc")
                nc.vector.reciprocal(rc, rs)
                sc = sm_pool.tile([P, 1], F32, tag="sc")
                nc.vector.tensor_mul(sc, rc, gam_sb)
                nc.vector.tensor_scalar_mul(st[bi]["a"][:, qi, :], a_f, sc)

            def emit_ATcombine(bi):
                lhsf = lhsf_pool.tile([P, CT, C1], BF16, name=f"lhsf{bi}")
                a_scaled = st[bi]["a"]
                for ct in range(CT):
                    ps_at = psT.tile([P, C], F32, tag="t", name=f"at{bi}")
                    for qi in range(QT):
                        nc.tensor.matmul(
                            ps_at[:, qi * P:(qi + 1) * P],
                            a_scaled[:, qi, ct * P:(ct + 1) * P], ident,
                            start=True, stop=True)
                    nc.vector.tensor_add(
                        out=lhsf[:, ct, :], in0=ps_at[:, :C1],
                        in1=wtf_sb[:, ct, :])
                st[bi]["lhsf"] = lhsf

            def emit_F_chunk(bi, qi, nch):
                lhsf = st[bi]["lhsf"]
                half = nch % 2
                if half == 0:
                    st[bi]["osb"] = osb_pool.tile([P, 2 * C], BF16, tag="o",
                                                  name=f"osb{bi}")
                o_sb = st[bi]["osb"]
                ps_o = psF.tile([P, C], F32, tag="po", name="ps_o")
                rhs = st[bi]["xb"][nch]
                for ct in range(CT):
                    nc.tensor.matmul(ps_o,
                                     lhsf[:, ct, qi * P:(qi + 1) * P],
                                     rhs[:, ct, :],
                                     start=(ct == 0), stop=(ct == CT - 1))
                oslice = o_sb[:, half * C:(half + 1) * C]
                if nch % 4 < 2:
                    nc.scalar.add(out=oslice, in_=ps_o,
                                  add=bq_sb[:, qi:qi + 1])
                else:
                    nc.vector.tensor_scalar_add(oslice, ps_o,
                                                bq_sb[:, qi:qi + 1])
                if half == 1:
                    nc.sync.dma_start(
                        out=out_r[bi, :, qi, (nch - 1) * C:(nch + 1) * C],
                        in_=o_sb)

            # ---------------- the schedule ----------------
            for nt in range(NT):
                emit_G_nt(0, nt)
            emit_s_reduce(0)            # DVE; fires as xb(b0) chunks land
            emit_G_evac(0)

            for nt in range(NT):
                emit_G_nt(1, nt)
                if nt == 4:
                    emit_G_lower(0)
                if nt == 10:
                    emit_srow(0)
                if nt == 14:
                    emit_WGq(0, 0)
                if nt == 18:
                    emit_WGq(0, 1)
                if nt == 24:
                    emit_ATcombine(0)
            emit_s_reduce(1)
            emit_G_evac(1)

            fseq = [(qi, nch) for qi in range(QT) for nch in range(NCH)]
            for g, (qi, nch) in enumerate(fseq):
                emit_F_chunk(0, qi, nch)
                if g == 2:
                    emit_G_lower(1)
                if g == 4:
                    emit_srow(1)
                if g == 6:
                    emit_WGq(1, 0)
                if g == 8:
                    emit_WGq(1, 1)
                if g == 12:
                    emit_ATcombine(1)
            for qi, nch in fseq:
                emit_F_chunk(1, qi, nch)
    nc.compile()
    return nc


_NC_CACHE = None


def _get_nc():
    global _NC_CACHE
    if _NC_CACHE is None:
        _NC_CACHE = build_nc()
    return _NC_CACHE


def make_in_maps(x, conv_w, conv_b, gamma):
    B = x.shape[0]
    x = np.asarray(x, dtype=np.float32)
    # c-partitioned bf16: [B, P, CT, HW]
    xb_full = np.ascontiguousarray(
        x.reshape(B, CT, P, HW).transpose(0, 2, 1, 3)).astype(
            ml_dtypes.bfloat16)
    # n-partitioned bf16: [B, P, NT, C]
    xt_full = np.ascontiguousarray(
        x.reshape(B, C, NT, P).transpose(0, 3, 2, 1)).astype(
            ml_dtypes.bfloat16)
    wm = conv_w.reshape(C1, C).astype(np.float32)
    wt_tiled = np.ascontiguousarray(
        wm.T.reshape(CT, P, C1).transpose(1, 0, 2))      # [P, CT, C1]
    b_np = conv_b.astype(np.float32)
    b_row = np.ascontiguousarray(b_np.reshape(1, C1)).astype(
        ml_dtypes.bfloat16)
    bq = np.ascontiguousarray(b_np.reshape(QT, P).T)     # [P, QT]
    gam = np.ascontiguousarray(
        np.broadcast_to(gamma.astype(np.float32).reshape(1, 1), (P, 1)))
    in_maps = []
    for ci in range(N_CORES):
        in_maps.append({
            "xt": np.ascontiguousarray(xt_full[NB * ci:NB * (ci + 1)]),
            "xb": np.ascontiguousarray(xb_full[NB * ci:NB * (ci + 1)]),
            "wt_f": wt_tiled,
            "b_row": b_row,
            "bq": bq,
            "gam": gam,
        })
    return in_maps


def kernel(x, conv_w, conv_b, gamma, trace=False):
    """Full inputs in, full output out. Shards batch over 8 NeuronCores."""
    nc = _get_nc()
    in_maps = make_in_maps(x, conv_w, conv_b, gamma)
    res = run_bass_kernel_spmd(nc, in_maps, core_ids=list(range(N_CORES)),
                               trace=trace)
    outs = [np.asarray(r["out"]).astype(np.float32).reshape(NB, C1, 64, 64)
            for r in res.results]
    full = np.concatenate(outs, axis=0)
    if trace:
        kernel.last_results = res
    return full


kernel.last_results = None
